# revision 7
# baseline (speedup 1.0000x reference)
"""GroupedQueryAttentionLayer on 8 trn2 NeuronCores (Bass/Tile, SPMD).

Sharding: data-parallel over query rows; no collectives. Core i handles
batch b = i//4, query rows q0 = (i%4)*512 .. +512. Each core recomputes
its batch's K/V projection; outputs are disjoint row-slices.

v2.1: fp8e4 DoubleRow matmuls everywhere (DR = 1.0 cyc/row on this HW;
the win is 2x contraction packing -> half the accumulation instructions).
Softmax exp split between ACT (exact exp, fp8 out; ACT does nothing else)
and DVE (Schraudolph affine -> int8 bits -> bitcast fp8). Weights scaled
x16 on host (fp8 subnormal avoidance), compensated in the exp scale
(1/2048), and post output scaled 1/256 at the final evict (Wpost x16 fp8,
residual added via a 256*I identity matmul). Reciprocal via partition-
spread; recip broadcast via gpsimd partition_broadcast (Pool engine).

Host layouts (fp8 = ml_dtypes.float8_e4m3):
  XT8  [128, 8, 2048]  X[b].T k-chunked      XTQ8 [128, 8, 512] q-slab cols
  XRES [128, 4, 1024]  bf16 residual rows    ID   [128, 128] bf16 256*I
  WQ8/WK8/WV8/WP8      k-chunked weights x16 (fp8)
  QTZ8 [128, 17, 512]  zeros (Q slots 2c+1 runtime-written)
  KTZ8 [128, 8, 2048]  zeros (K parity slots runtime-written)
Output OUT [4, 128, 1024] f32, OUT[sc, p, :] = row q0 + sc*128 + p.
"""

import math

import numpy as np
import ml_dtypes

BF16 = ml_dtypes.bfloat16
F8 = ml_dtypes.float8_e4m3

B, S, D = 2, 2048, 1024
HEADS, GROUPS, E = 16, 4, 64
NCORES = 8
CORES_PER_BATCH = NCORES // B
SLOC = B * S // NCORES
WSCALE = 16.0
EXPSC = (1.0 / math.sqrt(E)) / (WSCALE * WSCALE)
SCH_A = EXPSC * 8.0 / math.log(2.0)
SCH_B = 55.4
OSC = 1.0 / (WSCALE * WSCALE)  # post evict scale (atn x16 * wp x16)

# exp tcc-steps handled by DVE (rest on ACT)
DVE_EXP_STEPS = (0, 2, 4, 6, 9, 11, 13, 15)

_prog_cache = {}


def _build_program():
    from contextlib import ExitStack

    import concourse.bacc as bacc
    import concourse.tile as tile
    from concourse import mybir

    f32 = mybir.dt.float32
    b16 = mybir.dt.bfloat16
    f8 = mybir.dt.float8e4
    i8 = mybir.dt.int8
    Exp = mybir.ActivationFunctionType.Exp
    DR = mybir.MatmulPerfMode.DoubleRow
    MUL = mybir.AluOpType.mult
    ADD = mybir.AluOpType.add

    nc = bacc.Bacc("TRN2", target_bir_lowering=False)

    xt_d = nc.dram_tensor("XT8", [128, 8, S], f8, kind="ExternalInput")
    xtq_d = nc.dram_tensor("XTQ8", [128, 8, SLOC], f8, kind="ExternalInput")
    xres_d = nc.dram_tensor("XRES", [128, 4, D], b16, kind="ExternalInput")
    wq_d = nc.dram_tensor("WQ8", [128, 8, 1024], f8, kind="ExternalInput")
    wk_d = nc.dram_tensor("WK8", [128, 8, 256], f8, kind="ExternalInput")
    wv_d = nc.dram_tensor("WV8", [128, 8, 256], f8, kind="ExternalInput")
    wp_d = nc.dram_tensor("WP8", [128, 8, 1024], f8, kind="ExternalInput")
    ktz_d = nc.dram_tensor("KTZ8", [128, 8, S], f8, kind="ExternalInput")
    id_d = nc.dram_tensor("ID", [128, 128], b16, kind="ExternalInput")
    out_d = nc.dram_tensor("OUT", [4, 128, D], f32, kind="ExternalOutput")

    with tile.TileContext(nc) as tc, ExitStack() as ctx:
        consts = ctx.enter_context(tc.tile_pool(name="consts", bufs=1))
        work = ctx.enter_context(tc.tile_pool(name="work", bufs=2))
        psS = ctx.enter_context(tc.tile_pool(name="psS", bufs=2, space="PSUM"))
        psV = ctx.enter_context(tc.tile_pool(name="psV", bufs=1, space="PSUM"))
        psA = ctx.enter_context(tc.tile_pool(name="psA", bufs=2, space="PSUM"))

        xt = consts.tile([128, 8, S], f8)
        xtq = consts.tile([128, 8, SLOC], f8)
        xres = consts.tile([128, 4, D], b16)
        wq = consts.tile([128, 8, 1024], f8)
        wk = consts.tile([128, 8, 256], f8)
        wv = consts.tile([128, 8, 256], f8)
        wp = consts.tile([128, 8, 1024], f8)
        qt = consts.tile([128, 8, SLOC], f8)
        ktz = consts.tile([128, 8, S], f8)
        ksh = consts.tile([128, 2, S], f8)  # K eviction shadow
        id128 = consts.tile([128, 128], b16)
        vpr = consts.tile([128, 16, 4, 128], f8)  # V(64) | ones | zeros(63)
        atn = consts.tile([128, 8, SLOC], f8)
        rb0e = consts.tile([1, SLOC], b16)
        rb0o = consts.tile([1, SLOC], b16)
        warm = consts.tile([128, 8], f32)

        nc.vector.memset(warm[:], 0.0)
        nc.scalar.activation(warm[:], warm[:], Exp)  # exp table preload
        nc.gpsimd.memset(vpr[:, :, :, 64:65], 1.0)
        nc.gpsimd.memset(vpr[:, :, :, 65:128], 0.0)

        nc.sync.dma_start(out=wq[:], in_=wq_d[:])
        nc.sync.dma_start(out=xtq[:], in_=xtq_d[:])
        nc.sync.dma_start(out=wk[:], in_=wk_d[:])
        nc.sync.dma_start(out=xt[:, :, 0:512], in_=xt_d[:, :, 0:512])
        nc.sync.dma_start(out=ktz[:, 0:4, :], in_=ktz_d[:, 0:4, :])
        nc.sync.dma_start(out=wv[:], in_=wv_d[:])
        nc.sync.dma_start(out=xt[:, :, 512:1024], in_=xt_d[:, :, 512:1024])
        nc.sync.dma_start(out=xt[:, :, 1024:1536], in_=xt_d[:, :, 1024:1536])
        nc.sync.dma_start(out=xt[:, :, 1536:2048], in_=xt_d[:, :, 1536:2048])
        nc.sync.dma_start(out=ktz[:, 4:8, :], in_=ktz_d[:, 4:8, :])
        nc.sync.dma_start(out=wp[:], in_=wp_d[:])
        nc.sync.dma_start(out=xres[:], in_=xres_d[:])
        nc.sync.dma_start(out=id128[:], in_=id_d[:])

        def q_proj(c):
            ps = psA.tile([128, SLOC], f32, tag="pp")
            for u in range(4):
                nc.tensor.matmul(
                    ps[:],
                    lhsT=wq[:, 2 * u:2 * u + 2, c * 128:(c + 1) * 128],
                    rhs=xtq[:, 2 * u:2 * u + 2, :],
                    start=(u == 0), stop=(u == 3), perf_mode=DR,
                )
            nc.vector.tensor_copy(qt[:, c, :], ps[:])

        def k_proj_tile(gg, tb):
            """K groups (2gg, 2gg+1), t-block tb -> shadow -> ktz slots."""
            sl = slice(tb * 512, (tb + 1) * 512)
            ps = psA.tile([128, 512], f32, tag="pp")
            for u in range(4):
                nc.tensor.matmul(
                    ps[:],
                    lhsT=wk[:, 2 * u:2 * u + 2, gg * 128:(gg + 1) * 128],
                    rhs=xt[:, 2 * u:2 * u + 2, sl],
                    start=(u == 0), stop=(u == 3), perf_mode=DR,
                )
            nc.vector.tensor_copy(ksh[:, gg, sl], ps[:])
            ga, gb = 2 * gg, 2 * gg + 1
            nc.sync.dma_start(out=ktz[0:64, 2 * ga, sl], in_=ksh[0:64, gg, sl])
            nc.sync.dma_start(out=ktz[64:128, 2 * ga + 1, sl],
                              in_=ksh[0:64, gg, sl])
            nc.sync.dma_start(out=ktz[64:128, 2 * gb + 1, sl],
                              in_=ksh[64:128, gg, sl])
            nc.sync.dma_start(out=ktz[0:64, 2 * gb, sl],
                              in_=ksh[64:128, gg, sl])

        def v_proj(tcc):
            ps = psA.tile([128, 256], f32, tag="pp")
            for u in range(4):
                nc.tensor.matmul(
                    ps[:],
                    lhsT=xt[:, 2 * u:2 * u + 2, tcc * 128:(tcc + 1) * 128],
                    rhs=wv[:, 2 * u:2 * u + 2, :],
                    start=(u == 0), stop=(u == 3), perf_mode=DR,
                )
            nc.vector.tensor_copy(
                vpr[:, tcc, :, 0:64], ps.rearrange("p (g e) -> p g e", g=4)
            )

        exws = {}
        pvt = {}
        aun = {}

        def scores_exp(c, tcc):
            g = c // 2
            ps2 = psS.tile([128, 2, 512], f32, tag="sc", name="ps2")
            for j in range(2):
                nc.tensor.matmul(
                    ps2[:, j, :],
                    lhsT=ktz[:, 2 * g + j, tcc * 128:(tcc + 1) * 128],
                    rhs=qt[:, c, :], start=True, stop=True)
            u = tcc // 2
            if tcc % 2 == 0:
                exws[(c, u)] = work.tile([128, 2, 2, 512], i8, tag="exw",
                                         bufs=5, name="exw")
            exw = exws[(c, u)]
            dst = exw[:, tcc % 2, :, :]
            if tcc in DVE_EXP_STEPS:
                nc.vector.tensor_scalar(dst, ps2[:], SCH_A, SCH_B, MUL, ADD)
            else:
                nc.scalar.activation(dst.bitcast(f8), ps2[:], Exp, scale=EXPSC)

        def pv_mm(c, u):
            g = c // 2
            if u == 0:
                pvt[c] = (
                    psV.tile([128, 512], f32, tag="pve", name="pve"),
                    psV.tile([128, 512], f32, tag="pvo", name="pvo"),
                )
            pve, pvo = pvt[c]
            exw = exws.pop((c, u))
            for j, pv in ((0, pve), (1, pvo)):
                nc.tensor.matmul(
                    pv[:],
                    lhsT=vpr[:, 2 * u:2 * u + 2, g, :],
                    rhs=exw[:, :, j, :].bitcast(f8),
                    start=(u == 0), stop=(u == 7), perf_mode=DR,
                )

        def pv_evict(c, j):
            t = work.tile([65, 512], b16, tag="aun" + str(j), name="aun")
            nc.vector.tensor_copy(t[:], pvt[c][j][0:65, :])
            if j == 0:
                aun[c] = [t, None]
            else:
                aun[c][1] = t
                pvt.pop(c)

        def recip_spread(c, j):
            """den row 64 of te/to -> [64,8] -> recip -> rb0 row 0."""
            t = aun[c][j]
            rb0 = rb0e if j == 0 else rb0o
            dsp = work.tile([64, 8], b16, tag="dsp")
            nc.sync.dma_start(
                out=dsp[:, None, :],
                in_=t[64:65, :].rearrange("p (a b) -> p a b", a=64),
            )
            rsp = work.tile([64, 8], b16, tag="rsp")
            with nc.allow_low_precision(reason="bf16 softmax recip"):
                nc.vector.reciprocal(rsp[:], dsp[:])
            nc.sync.dma_start(
                out=rb0[0:1, :].rearrange("p (a b) -> p a b", a=64),
                in_=rsp[:, None, :],
            )

        bcss = {}

        def bcast(c, j):
            rb0 = rb0e if j == 0 else rb0o
            bcs = work.tile([64, 512], b16, tag="bcs" + str(j), name="bcs")
            nc.gpsimd.partition_broadcast(bcs[:], rb0[0:1, :], channels=64)
            bcss[(c, j)] = bcs

        def norm_head(c, j):
            t = aun[c][j]
            bcs = bcss.pop((c, j))
            if j == 0:
                nc.gpsimd.tensor_tensor(atn[0:64, c, :], t[0:64, :], bcs[:],
                                        MUL)
            else:
                so = work.tile([64, 512], f8, tag="so", name="so")
                nc.gpsimd.tensor_tensor(so[:], t[0:64, :], bcs[:], MUL)
                nc.sync.dma_start(out=atn[64:128, c, :], in_=so[:])
                aun.pop(c)

        def tail(b, step):
            if step == 0:
                pv_mm(b, 7)
            elif step == 1:
                pv_evict(b, 0)
            elif step == 2:
                pv_evict(b, 1)
                recip_spread(b, 0)
            elif step == 3:
                recip_spread(b, 1)
                bcast(b, 0)
            elif step == 4:
                bcast(b, 1)
            elif step == 5:
                norm_head(b, 0)
            elif step == 6:
                norm_head(b, 1)

        def post_tile(i):
            sc, dc = i // 2, i % 2
            dsl = slice(dc * 512, (dc + 1) * 512)
            pp = psA.tile([128, 512], f32, tag="pp")
            for u in range(4):
                nc.tensor.matmul(pp[:],
                                 lhsT=atn[:, 2 * u:2 * u + 2,
                                          sc * 128:(sc + 1) * 128],
                                 rhs=wp[:, 2 * u:2 * u + 2, dsl],
                                 start=(u == 0), stop=False, perf_mode=DR)
            nc.tensor.matmul(pp[:], lhsT=id128[:], rhs=xres[:, sc, dsl],
                             start=False, stop=True)
            ores = work.tile([128, 512], f32, tag="or", bufs=3, name="ores")
            nc.scalar.mul(ores[:], pp[:], OSC)
            nc.sync.dma_start(out=out_d[sc, :, dsl], in_=ores[:])

        def pair_block(c):
            for step in range(16):
                if c < 8:
                    scores_exp(c, step)
                    if step >= 3 and step % 2 == 1:
                        pv_mm(c, (step - 3) // 2)
                if c >= 1:
                    tail(c - 1, step)
                if c == 0 and step < 12:
                    v_proj(step + 4)
                if c == 1 and step in (2, 6, 10, 14):
                    k_proj_tile(1, (step - 2) // 4)
                if c <= 5 and step == 9:
                    q_proj(c + 2)

        q_proj(0)
        q_proj(1)
        for tb in range(4):
            k_proj_tile(0, tb)
        for tcc in range(4):
            v_proj(tcc)
        for c in range(8):
            pair_block(c)
        pair_block(8)  # drain: tail of pair 7
        for i in range(8):
            post_tile(i)

    nc.compile()
    return nc


def get_program():
    if "nc" not in _prog_cache:
        _prog_cache["nc"] = _build_program()
    return _prog_cache["nc"]


def _chunk128(a):
    n = a.shape[1]
    return np.ascontiguousarray(a.reshape(8, 128, n).transpose(1, 0, 2))


def make_in_maps(X, Wq, Wk, Wv, Wpost):
    X = np.asarray(X, dtype=np.float32)
    wq_p = _chunk128(np.asarray(Wq, dtype=np.float32) * WSCALE).astype(F8)
    wk_p = _chunk128(np.asarray(Wk, dtype=np.float32) * WSCALE).astype(F8)
    wv_p = _chunk128(np.asarray(Wv, dtype=np.float32) * WSCALE).astype(F8)
    wp_p = _chunk128(np.asarray(Wpost, dtype=np.float32) * WSCALE).astype(F8)
    ktz = np.zeros((128, 8, S), dtype=F8)
    idm = (np.eye(128, dtype=np.float32) * (WSCALE * WSCALE)).astype(BF16)

    xt_b = []
    for b in range(B):
        xt_b.append(_chunk128(np.ascontiguousarray(X[b].T)).astype(F8))

    in_maps = []
    for core in range(NCORES):
        b = core // CORES_PER_BATCH
        q0 = (core % CORES_PER_BATCH) * SLOC
        xt = xt_b[b]
        xres = np.ascontiguousarray(
            X[b, q0:q0 + SLOC].reshape(4, 128, D).transpose(1, 0, 2)
        ).astype(BF16)
        in_maps.append(
            {
                "XT8": xt,
                "XTQ8": np.ascontiguousarray(xt[:, :, q0:q0 + SLOC]),
                "XRES": xres,
                "WQ8": wq_p,
                "WK8": wk_p,
                "WV8": wv_p,
                "WP8": wp_p,
                "KTZ8": ktz,
                "ID": idm,
            }
        )
    return in_maps


def assemble_output(results):
    out = np.empty((B, S, D), dtype=np.float32)
    for core, r in enumerate(results):
        b = core // CORES_PER_BATCH
        q0 = (core % CORES_PER_BATCH) * SLOC
        out[b, q0:q0 + SLOC] = np.asarray(r["OUT"]).reshape(SLOC, D)
    return out


def kernel(X, Wq, Wk, Wv, Wpost, _trace=False):
    from concourse.bass_utils import run_bass_kernel_spmd

    nc = get_program()
    in_maps = make_in_maps(X, Wq, Wk, Wv, Wpost)
    res = run_bass_kernel_spmd(nc, in_maps, core_ids=list(range(NCORES)), trace=_trace)
    out = assemble_output(res.results)
    if _trace:
        return out, res
    return out


# revision 8
# speedup vs baseline: 1.1237x; 1.1237x over previous
"""GroupedQueryAttentionLayer on 8 trn2 NeuronCores (Bass/Tile, SPMD).

Sharding: data-parallel over query rows; no collectives. Core i handles
batch b = i//4, query rows q0 = (i%4)*512 .. +512. Each core recomputes
its batch's K/V projection; outputs are disjoint row-slices.

v2.1: fp8e4 DoubleRow matmuls everywhere (DR = 1.0 cyc/row on this HW;
the win is 2x contraction packing -> half the accumulation instructions).
Softmax exp split between ACT (exact exp, fp8 out; ACT does nothing else)
and DVE (Schraudolph affine -> int8 bits -> bitcast fp8). Weights scaled
x16 on host (fp8 subnormal avoidance), compensated in the exp scale
(1/2048), and post output scaled 1/256 at the final evict (Wpost x16 fp8,
residual added via a 256*I identity matmul). Reciprocal via partition-
spread; recip broadcast via gpsimd partition_broadcast (Pool engine).

Host layouts (fp8 = ml_dtypes.float8_e4m3):
  XT8  [128, 8, 2048]  X[b].T k-chunked      XTQ8 [128, 8, 512] q-slab cols
  XRES [128, 4, 1024]  bf16 residual rows    ID   [128, 128] bf16 256*I
  WQ8/WK8/WV8/WP8      k-chunked weights x16 (fp8)
  QTZ8 [128, 17, 512]  zeros (Q slots 2c+1 runtime-written)
  KTZ8 [128, 8, 2048]  zeros (K parity slots runtime-written)
Output OUT [4, 128, 1024] f32, OUT[sc, p, :] = row q0 + sc*128 + p.
"""

import math

import numpy as np
import ml_dtypes

BF16 = ml_dtypes.bfloat16
F8 = ml_dtypes.float8_e4m3

B, S, D = 2, 2048, 1024
HEADS, GROUPS, E = 16, 4, 64
NCORES = 8
CORES_PER_BATCH = NCORES // B
SLOC = B * S // NCORES
WSCALE = 16.0
EXPSC = (1.0 / math.sqrt(E)) / (WSCALE * WSCALE)
SCH_A = EXPSC * 8.0 / math.log(2.0)
SCH_B = 55.4
OSC = 1.0 / (WSCALE * WSCALE)  # post evict scale (atn x16 * wp x16)

# exp tcc-steps handled by DVE (rest on ACT)
DVE_EXP_STEPS = (0, 3, 6, 8, 11, 14)

_prog_cache = {}


def _build_program():
    from contextlib import ExitStack

    import concourse.bacc as bacc
    import concourse.tile as tile
    from concourse import mybir

    f32 = mybir.dt.float32
    b16 = mybir.dt.bfloat16
    f8 = mybir.dt.float8e4
    i8 = mybir.dt.int8
    Exp = mybir.ActivationFunctionType.Exp
    DR = mybir.MatmulPerfMode.DoubleRow
    MUL = mybir.AluOpType.mult
    ADD = mybir.AluOpType.add

    nc = bacc.Bacc("TRN2", target_bir_lowering=False)

    xt_d = nc.dram_tensor("XT8", [128, 8, S], f8, kind="ExternalInput")
    xtq_d = nc.dram_tensor("XTQ8", [128, 8, SLOC], f8, kind="ExternalInput")
    xres_d = nc.dram_tensor("XRES", [128, 4, D], b16, kind="ExternalInput")
    wq_d = nc.dram_tensor("WQ8", [128, 8, 1024], f8, kind="ExternalInput")
    wk_d = nc.dram_tensor("WK8", [128, 8, 256], f8, kind="ExternalInput")
    wv_d = nc.dram_tensor("WV8", [128, 8, 256], f8, kind="ExternalInput")
    wp_d = nc.dram_tensor("WP8", [128, 8, 1024], f8, kind="ExternalInput")
    ktz_d = nc.dram_tensor("KTZ8", [128, 8, S], f8, kind="ExternalInput")
    id_d = nc.dram_tensor("ID", [128, 128], b16, kind="ExternalInput")
    out_d = nc.dram_tensor("OUT", [4, 128, D], f32, kind="ExternalOutput")

    with tile.TileContext(nc) as tc, ExitStack() as ctx:
        consts = ctx.enter_context(tc.tile_pool(name="consts", bufs=1))
        work = ctx.enter_context(tc.tile_pool(name="work", bufs=2))
        psS = ctx.enter_context(tc.tile_pool(name="psS", bufs=2, space="PSUM"))
        psV = ctx.enter_context(tc.tile_pool(name="psV", bufs=1, space="PSUM"))
        psA = ctx.enter_context(tc.tile_pool(name="psA", bufs=2, space="PSUM"))

        xt = consts.tile([128, 8, S], f8)
        xtq = consts.tile([128, 8, SLOC], f8)
        xres = consts.tile([128, 4, D], b16)
        wq = consts.tile([128, 8, 1024], f8)
        wk = consts.tile([128, 8, 256], f8)
        wv = consts.tile([128, 8, 256], f8)
        wp = consts.tile([128, 8, 1024], f8)
        qt = consts.tile([128, 8, SLOC], f8)
        ktz = consts.tile([128, 8, S], f8)
        ksh = consts.tile([128, 2, S], f8)  # K eviction shadow
        id128 = consts.tile([128, 128], b16)
        vpr = consts.tile([128, 16, 4, 128], f8)  # V(64) | ones | zeros(63)
        atn = consts.tile([128, 8, SLOC], f8)
        rb0e = consts.tile([1, SLOC], b16)
        rb0o = consts.tile([1, SLOC], b16)
        warm = consts.tile([128, 8], f32)

        nc.vector.memset(warm[:], 0.0)
        nc.scalar.activation(warm[:], warm[:], Exp)  # exp table preload
        nc.gpsimd.memset(vpr[:, :, :, 64:65], 1.0)
        nc.gpsimd.memset(vpr[:, :, :, 65:128], 0.0)

        nc.sync.dma_start(out=wq[:], in_=wq_d[:])
        nc.sync.dma_start(out=xtq[:], in_=xtq_d[:])
        nc.sync.dma_start(out=wk[:], in_=wk_d[:])
        nc.sync.dma_start(out=xt[:, :, 0:512], in_=xt_d[:, :, 0:512])
        nc.sync.dma_start(out=ktz[:, 0:4, :], in_=ktz_d[:, 0:4, :])
        nc.sync.dma_start(out=wv[:], in_=wv_d[:])
        nc.sync.dma_start(out=xt[:, :, 512:1024], in_=xt_d[:, :, 512:1024])
        nc.sync.dma_start(out=xt[:, :, 1024:1536], in_=xt_d[:, :, 1024:1536])
        nc.sync.dma_start(out=xt[:, :, 1536:2048], in_=xt_d[:, :, 1536:2048])
        nc.sync.dma_start(out=ktz[:, 4:8, :], in_=ktz_d[:, 4:8, :])
        nc.sync.dma_start(out=wp[:], in_=wp_d[:])
        nc.sync.dma_start(out=xres[:], in_=xres_d[:])
        nc.sync.dma_start(out=id128[:], in_=id_d[:])

        def q_proj(c):
            ps = psA.tile([128, SLOC], f32, tag="pp")
            for u in range(4):
                nc.tensor.matmul(
                    ps[:],
                    lhsT=wq[:, 2 * u:2 * u + 2, c * 128:(c + 1) * 128],
                    rhs=xtq[:, 2 * u:2 * u + 2, :],
                    start=(u == 0), stop=(u == 3), perf_mode=DR,
                )
            nc.vector.tensor_copy(qt[:, c, :], ps[:])

        def k_proj_tile(gg, tb):
            """K groups (2gg, 2gg+1), t-block tb -> shadow -> ktz slots."""
            sl = slice(tb * 512, (tb + 1) * 512)
            ps = psA.tile([128, 512], f32, tag="pp")
            for u in range(4):
                nc.tensor.matmul(
                    ps[:],
                    lhsT=wk[:, 2 * u:2 * u + 2, gg * 128:(gg + 1) * 128],
                    rhs=xt[:, 2 * u:2 * u + 2, sl],
                    start=(u == 0), stop=(u == 3), perf_mode=DR,
                )
            nc.vector.tensor_copy(ksh[:, gg, sl], ps[:])
            ga, gb = 2 * gg, 2 * gg + 1
            nc.sync.dma_start(out=ktz[0:64, 2 * ga, sl], in_=ksh[0:64, gg, sl])
            nc.sync.dma_start(out=ktz[64:128, 2 * ga + 1, sl],
                              in_=ksh[0:64, gg, sl])
            nc.sync.dma_start(out=ktz[64:128, 2 * gb + 1, sl],
                              in_=ksh[64:128, gg, sl])
            nc.sync.dma_start(out=ktz[0:64, 2 * gb, sl],
                              in_=ksh[64:128, gg, sl])

        def v_proj(tcc):
            ps = psA.tile([128, 256], f32, tag="pp")
            for u in range(4):
                nc.tensor.matmul(
                    ps[:],
                    lhsT=xt[:, 2 * u:2 * u + 2, tcc * 128:(tcc + 1) * 128],
                    rhs=wv[:, 2 * u:2 * u + 2, :],
                    start=(u == 0), stop=(u == 3), perf_mode=DR,
                )
            nc.vector.tensor_copy(
                vpr[:, tcc, :, 0:64], ps.rearrange("p (g e) -> p g e", g=4)
            )

        exws = {}
        pvt = {}
        aun = {}

        def scores_exp(c, tcc):
            g = c // 2
            ps2 = psS.tile([128, 2, 512], f32, tag="sc", name="ps2")
            for j in range(2):
                nc.tensor.matmul(
                    ps2[:, j, :],
                    lhsT=ktz[:, 2 * g + j, tcc * 128:(tcc + 1) * 128],
                    rhs=qt[:, c, :], start=True, stop=True)
            u = tcc // 2
            if tcc % 2 == 0:
                exws[(c, u)] = work.tile([128, 2, 2, 512], i8, tag="exw",
                                         bufs=5, name="exw")
            exw = exws[(c, u)]
            dst = exw[:, tcc % 2, :, :]
            if tcc in DVE_EXP_STEPS:
                nc.vector.tensor_scalar(dst, ps2[:], SCH_A, SCH_B, MUL, ADD)
            else:
                nc.scalar.activation(dst.bitcast(f8), ps2[:], Exp, scale=EXPSC)

        def pv_mm(c, u):
            g = c // 2
            if u == 0:
                pvt[c] = (
                    psV.tile([128, 512], f32, tag="pve", name="pve"),
                    psV.tile([128, 512], f32, tag="pvo", name="pvo"),
                )
            pve, pvo = pvt[c]
            exw = exws.pop((c, u))
            for j, pv in ((0, pve), (1, pvo)):
                nc.tensor.matmul(
                    pv[:],
                    lhsT=vpr[:, 2 * u:2 * u + 2, g, :],
                    rhs=exw[:, :, j, :].bitcast(f8),
                    start=(u == 0), stop=(u == 7), perf_mode=DR,
                )

        def pv_evict(c, j):
            t = work.tile([65, 512], b16, tag="aun" + str(j), name="aun")
            nc.vector.tensor_copy(t[:], pvt[c][j][0:65, :])
            if j == 0:
                aun[c] = [t, None]
            else:
                aun[c][1] = t
                pvt.pop(c)

        def recip_spread(c, j):
            """den row 64 of te/to -> [64,8] -> recip -> rb0 row 0."""
            t = aun[c][j]
            rb0 = rb0e if j == 0 else rb0o
            dsp = work.tile([64, 8], b16, tag="dsp")
            nc.sync.dma_start(
                out=dsp[:, None, :],
                in_=t[64:65, :].rearrange("p (a b) -> p a b", a=64),
            )
            rsp = work.tile([64, 8], b16, tag="rsp")
            with nc.allow_low_precision(reason="bf16 softmax recip"):
                nc.vector.reciprocal(rsp[:], dsp[:])
            nc.sync.dma_start(
                out=rb0[0:1, :].rearrange("p (a b) -> p a b", a=64),
                in_=rsp[:, None, :],
            )

        bcss = {}

        def bcast(c, j):
            rb0 = rb0e if j == 0 else rb0o
            bcs = work.tile([64, 512], b16, tag="bcs" + str(j), name="bcs")
            nc.gpsimd.partition_broadcast(bcs[:], rb0[0:1, :], channels=64)
            bcss[(c, j)] = bcs

        def norm_head(c, j):
            t = aun[c][j]
            bcs = bcss.pop((c, j))
            if j == 0:
                nc.gpsimd.tensor_tensor(atn[0:64, c, :], t[0:64, :], bcs[:],
                                        MUL)
            else:
                so = work.tile([64, 512], f8, tag="so", name="so")
                nc.gpsimd.tensor_tensor(so[:], t[0:64, :], bcs[:], MUL)
                nc.sync.dma_start(out=atn[64:128, c, :], in_=so[:])
                aun.pop(c)

        def tail(b, step):
            if step == 0:
                pv_mm(b, 7)
            elif step == 1:
                pv_evict(b, 0)
            elif step == 2:
                pv_evict(b, 1)
                recip_spread(b, 0)
            elif step == 3:
                recip_spread(b, 1)
                bcast(b, 0)
            elif step == 4:
                bcast(b, 1)
            elif step == 5:
                norm_head(b, 0)
            elif step == 6:
                norm_head(b, 1)

        def post_tile(i):
            sc, dc = i // 2, i % 2
            dsl = slice(dc * 512, (dc + 1) * 512)
            pp = psA.tile([128, 512], f32, tag="pp")
            for u in range(4):
                nc.tensor.matmul(pp[:],
                                 lhsT=atn[:, 2 * u:2 * u + 2,
                                          sc * 128:(sc + 1) * 128],
                                 rhs=wp[:, 2 * u:2 * u + 2, dsl],
                                 start=(u == 0), stop=False, perf_mode=DR)
            nc.tensor.matmul(pp[:], lhsT=id128[:], rhs=xres[:, sc, dsl],
                             start=False, stop=True)
            ores = work.tile([128, 512], f32, tag="or", bufs=3, name="ores")
            nc.scalar.mul(ores[:], pp[:], OSC)
            nc.sync.dma_start(out=out_d[sc, :, dsl], in_=ores[:])

        def pair_block(c):
            for step in range(16):
                if c < 8:
                    scores_exp(c, step)
                    if step >= 3 and step % 2 == 1:
                        pv_mm(c, (step - 3) // 2)
                if c >= 1:
                    tail(c - 1, step)
                if c == 0 and step < 12:
                    v_proj(step + 4)
                if c == 1 and step in (2, 6, 10, 14):
                    k_proj_tile(1, (step - 2) // 4)
                if c <= 5 and step == 9:
                    q_proj(c + 2)

        q_proj(0)
        q_proj(1)
        for tb in range(4):
            k_proj_tile(0, tb)
        for tcc in range(4):
            v_proj(tcc)
        for c in range(8):
            pair_block(c)
        pair_block(8)  # drain: tail of pair 7
        for i in range(8):
            post_tile(i)

    nc.compile()
    return nc


def get_program():
    if "nc" not in _prog_cache:
        _prog_cache["nc"] = _build_program()
    return _prog_cache["nc"]


def _chunk128(a):
    n = a.shape[1]
    return np.ascontiguousarray(a.reshape(8, 128, n).transpose(1, 0, 2))


def make_in_maps(X, Wq, Wk, Wv, Wpost):
    X = np.asarray(X, dtype=np.float32)
    wq_p = _chunk128(np.asarray(Wq, dtype=np.float32) * WSCALE).astype(F8)
    wk_p = _chunk128(np.asarray(Wk, dtype=np.float32) * WSCALE).astype(F8)
    wv_p = _chunk128(np.asarray(Wv, dtype=np.float32) * WSCALE).astype(F8)
    wp_p = _chunk128(np.asarray(Wpost, dtype=np.float32) * WSCALE).astype(F8)
    ktz = np.zeros((128, 8, S), dtype=F8)
    idm = (np.eye(128, dtype=np.float32) * (WSCALE * WSCALE)).astype(BF16)

    xt_b = []
    for b in range(B):
        xt_b.append(_chunk128(np.ascontiguousarray(X[b].T)).astype(F8))

    in_maps = []
    for core in range(NCORES):
        b = core // CORES_PER_BATCH
        q0 = (core % CORES_PER_BATCH) * SLOC
        xt = xt_b[b]
        xres = np.ascontiguousarray(
            X[b, q0:q0 + SLOC].reshape(4, 128, D).transpose(1, 0, 2)
        ).astype(BF16)
        in_maps.append(
            {
                "XT8": xt,
                "XTQ8": np.ascontiguousarray(xt[:, :, q0:q0 + SLOC]),
                "XRES": xres,
                "WQ8": wq_p,
                "WK8": wk_p,
                "WV8": wv_p,
                "WP8": wp_p,
                "KTZ8": ktz,
                "ID": idm,
            }
        )
    return in_maps


def assemble_output(results):
    out = np.empty((B, S, D), dtype=np.float32)
    for core, r in enumerate(results):
        b = core // CORES_PER_BATCH
        q0 = (core % CORES_PER_BATCH) * SLOC
        out[b, q0:q0 + SLOC] = np.asarray(r["OUT"]).reshape(SLOC, D)
    return out


def kernel(X, Wq, Wk, Wv, Wpost, _trace=False):
    from concourse.bass_utils import run_bass_kernel_spmd

    nc = get_program()
    in_maps = make_in_maps(X, Wq, Wk, Wv, Wpost)
    res = run_bass_kernel_spmd(nc, in_maps, core_ids=list(range(NCORES)), trace=_trace)
    out = assemble_output(res.results)
    if _trace:
        return out, res
    return out


# revision 9
# speedup vs baseline: 1.9151x; 1.7043x over previous
"""GroupedQueryAttentionLayer on 8 trn2 NeuronCores (Bass/Tile, SPMD).

Sharding: data-parallel over query rows; no collectives. Core i handles
batch b = i//4, query rows q0 = (i%4)*512 .. +512. Each core recomputes
its batch's K/V projection; outputs are disjoint row-slices.

v2.1: fp8e4 DoubleRow matmuls everywhere (DR = 1.0 cyc/row on this HW;
the win is 2x contraction packing -> half the accumulation instructions).
Softmax exp split between ACT (exact exp, fp8 out; ACT does nothing else)
and DVE (Schraudolph affine -> int8 bits -> bitcast fp8). Weights scaled
x16 on host (fp8 subnormal avoidance), compensated in the exp scale
(1/2048), and post output scaled 1/256 at the final evict (Wpost x16 fp8,
residual added via a 256*I identity matmul). Reciprocal via partition-
spread; recip broadcast via gpsimd partition_broadcast (Pool engine).

Host layouts (fp8 = ml_dtypes.float8_e4m3):
  XT8  [128, 8, 2048]  X[b].T k-chunked      XTQ8 [128, 8, 512] q-slab cols
  XRES [128, 4, 1024]  bf16 residual rows    ID   [128, 128] bf16 256*I
  WQ8/WK8/WV8/WP8      k-chunked weights x16 (fp8)
  QTZ8 [128, 17, 512]  zeros (Q slots 2c+1 runtime-written)
  KTZ8 [128, 8, 2048]  zeros (K parity slots runtime-written)
Output OUT [4, 128, 1024] f32, OUT[sc, p, :] = row q0 + sc*128 + p.
"""

import math

import numpy as np
import ml_dtypes

BF16 = ml_dtypes.bfloat16
F8 = ml_dtypes.float8_e4m3

B, S, D = 2, 2048, 1024
HEADS, GROUPS, E = 16, 4, 64
NCORES = 8
CORES_PER_BATCH = NCORES // B
SLOC = B * S // NCORES
WSCALE = 16.0
EXPSC = (1.0 / math.sqrt(E)) / (WSCALE * WSCALE)
SCH_A = EXPSC * 8.0 / math.log(2.0)
SCH_B = 55.4
OSC = 1.0 / (WSCALE * WSCALE)  # post evict scale (atn x16 * wp x16)

# exp tcc-steps handled by DVE (rest on ACT)
DVE_EXP_STEPS = (0, 3, 6, 8, 11, 14)

_prog_cache = {}


def _build_program():
    from contextlib import ExitStack

    import concourse.bacc as bacc
    import concourse.tile as tile
    from concourse import mybir

    f32 = mybir.dt.float32
    b16 = mybir.dt.bfloat16
    f8 = mybir.dt.float8e4
    i8 = mybir.dt.int8
    Exp = mybir.ActivationFunctionType.Exp
    DR = mybir.MatmulPerfMode.DoubleRow
    MUL = mybir.AluOpType.mult
    ADD = mybir.AluOpType.add

    nc = bacc.Bacc("TRN2", target_bir_lowering=False)

    xt_d = nc.dram_tensor("XT8", [128, 8, S], f8, kind="ExternalInput")
    xtq_d = nc.dram_tensor("XTQ8", [128, 8, SLOC], f8, kind="ExternalInput")
    xres_d = nc.dram_tensor("XRES", [128, 4, D], b16, kind="ExternalInput")
    wq_d = nc.dram_tensor("WQ8", [128, 8, 1024], f8, kind="ExternalInput")
    wk_d = nc.dram_tensor("WK8", [128, 8, 256], f8, kind="ExternalInput")
    wv_d = nc.dram_tensor("WV8", [128, 8, 256], f8, kind="ExternalInput")
    wp_d = nc.dram_tensor("WP8", [128, 8, 1024], f8, kind="ExternalInput")
    ktz_d = nc.dram_tensor("KTZ8", [128, 8, S], f8, kind="ExternalInput")
    id_d = nc.dram_tensor("ID", [128, 128], b16, kind="ExternalInput")
    out_d = nc.dram_tensor("OUT", [4, 128, D], f32, kind="ExternalOutput")

    with tile.TileContext(nc) as tc, ExitStack() as ctx:
        consts = ctx.enter_context(tc.tile_pool(name="consts", bufs=1))
        work = ctx.enter_context(tc.tile_pool(name="work", bufs=2))
        psS = ctx.enter_context(tc.tile_pool(name="psS", bufs=2, space="PSUM"))
        psV = ctx.enter_context(tc.tile_pool(name="psV", bufs=1, space="PSUM"))
        psA = ctx.enter_context(tc.tile_pool(name="psA", bufs=2, space="PSUM"))

        xt = consts.tile([128, 8, S], f8)
        xtq = consts.tile([128, 8, SLOC], f8)
        xres = consts.tile([128, 4, D], b16)
        wq = consts.tile([128, 8, 1024], f8)
        wk = consts.tile([128, 8, 256], f8)
        wv = consts.tile([128, 8, 256], f8)
        wp = consts.tile([128, 8, 1024], f8)
        qt = consts.tile([128, 8, SLOC], f8)
        ktz = consts.tile([128, 8, S], f8)
        ksh = consts.tile([128, 2, S], f8)  # K eviction shadow
        id128 = consts.tile([128, 128], b16)
        vpr = consts.tile([128, 16, 4, 128], f8)  # V(64) | ones | zeros(63)
        atn = consts.tile([128, 8, SLOC], f8)
        rb0e = consts.tile([1, SLOC], b16)
        rb0o = consts.tile([1, SLOC], b16)
        warm = consts.tile([128, 8], f32)

        nc.vector.memset(warm[:], 0.0)
        nc.scalar.activation(warm[:], warm[:], Exp)  # exp table preload
        nc.vector.memset(vpr[:, :, :, 64:65], 1.0)
        nc.vector.memset(vpr[:, :, :, 65:128], 0.0)

        nc.sync.dma_start(out=wq[:], in_=wq_d[:])
        nc.sync.dma_start(out=xtq[:], in_=xtq_d[:])
        nc.sync.dma_start(out=wk[:], in_=wk_d[:])
        nc.sync.dma_start(out=xt[:, :, 0:512], in_=xt_d[:, :, 0:512])
        nc.sync.dma_start(out=ktz[:, 0:4, :], in_=ktz_d[:, 0:4, :])
        nc.sync.dma_start(out=wv[:], in_=wv_d[:])
        nc.sync.dma_start(out=xt[:, :, 512:1024], in_=xt_d[:, :, 512:1024])
        nc.sync.dma_start(out=xt[:, :, 1024:1536], in_=xt_d[:, :, 1024:1536])
        nc.sync.dma_start(out=xt[:, :, 1536:2048], in_=xt_d[:, :, 1536:2048])
        nc.sync.dma_start(out=ktz[:, 4:8, :], in_=ktz_d[:, 4:8, :])
        nc.sync.dma_start(out=wp[:], in_=wp_d[:])
        nc.sync.dma_start(out=xres[:], in_=xres_d[:])
        nc.sync.dma_start(out=id128[:], in_=id_d[:])

        def q_proj(c):
            ps = psA.tile([128, SLOC], f32, tag="pp")
            for u in range(4):
                nc.tensor.matmul(
                    ps[:],
                    lhsT=wq[:, 2 * u:2 * u + 2, c * 128:(c + 1) * 128],
                    rhs=xtq[:, 2 * u:2 * u + 2, :],
                    start=(u == 0), stop=(u == 3), perf_mode=DR,
                )
            nc.vector.tensor_copy(qt[:, c, :], ps[:])

        def k_proj_tile(gg, tb):
            """K groups (2gg, 2gg+1), t-block tb -> shadow -> ktz slots."""
            sl = slice(tb * 512, (tb + 1) * 512)
            ps = psA.tile([128, 512], f32, tag="pp")
            for u in range(4):
                nc.tensor.matmul(
                    ps[:],
                    lhsT=wk[:, 2 * u:2 * u + 2, gg * 128:(gg + 1) * 128],
                    rhs=xt[:, 2 * u:2 * u + 2, sl],
                    start=(u == 0), stop=(u == 3), perf_mode=DR,
                )
            nc.vector.tensor_copy(ksh[:, gg, sl], ps[:])
            ga, gb = 2 * gg, 2 * gg + 1
            nc.gpsimd.dma_start(out=ktz[0:64, 2 * ga, sl], in_=ksh[0:64, gg, sl])
            nc.gpsimd.dma_start(out=ktz[64:128, 2 * ga + 1, sl],
                                in_=ksh[0:64, gg, sl])
            nc.gpsimd.dma_start(out=ktz[64:128, 2 * gb + 1, sl],
                                in_=ksh[64:128, gg, sl])
            nc.gpsimd.dma_start(out=ktz[0:64, 2 * gb, sl],
                                in_=ksh[64:128, gg, sl])

        def v_proj(tcc):
            ps = psA.tile([128, 256], f32, tag="pp")
            for u in range(4):
                nc.tensor.matmul(
                    ps[:],
                    lhsT=xt[:, 2 * u:2 * u + 2, tcc * 128:(tcc + 1) * 128],
                    rhs=wv[:, 2 * u:2 * u + 2, :],
                    start=(u == 0), stop=(u == 3), perf_mode=DR,
                )
            nc.vector.tensor_copy(
                vpr[:, tcc, :, 0:64], ps.rearrange("p (g e) -> p g e", g=4)
            )

        exws = {}
        pvt = {}
        aun = {}

        def scores_exp(c, tcc):
            g = c // 2
            ps2 = psS.tile([128, 2, 512], f32, tag="sc", name="ps2")
            for j in range(2):
                nc.tensor.matmul(
                    ps2[:, j, :],
                    lhsT=ktz[:, 2 * g + j, tcc * 128:(tcc + 1) * 128],
                    rhs=qt[:, c, :], start=True, stop=True)
            u = tcc // 2
            if tcc % 2 == 0:
                exws[(c, u)] = work.tile([128, 2, 2, 512], i8, tag="exw",
                                         bufs=5, name="exw")
            exw = exws[(c, u)]
            dst = exw[:, tcc % 2, :, :]
            if tcc in DVE_EXP_STEPS:
                nc.vector.tensor_scalar(dst, ps2[:], SCH_A, SCH_B, MUL, ADD)
            else:
                nc.scalar.activation(dst.bitcast(f8), ps2[:], Exp, scale=EXPSC)

        def pv_mm(c, u):
            g = c // 2
            if u == 0:
                pvt[c] = (
                    psV.tile([128, 512], f32, tag="pve", name="pve"),
                    psV.tile([128, 512], f32, tag="pvo", name="pvo"),
                )
            pve, pvo = pvt[c]
            exw = exws.pop((c, u))
            for j, pv in ((0, pve), (1, pvo)):
                nc.tensor.matmul(
                    pv[:],
                    lhsT=vpr[:, 2 * u:2 * u + 2, g, :],
                    rhs=exw[:, :, j, :].bitcast(f8),
                    start=(u == 0), stop=(u == 7), perf_mode=DR,
                )

        def pv_evict(c, j):
            t = work.tile([65, 512], b16, tag="aun" + str(j), name="aun")
            nc.vector.tensor_copy(t[:], pvt[c][j][0:65, :])
            if j == 0:
                aun[c] = [t, None]
            else:
                aun[c][1] = t
                pvt.pop(c)

        def recip_spread(c, j):
            """den row 64 of te/to -> [64,8] -> recip -> rb0 row 0."""
            t = aun[c][j]
            rb0 = rb0e if j == 0 else rb0o
            dsp = work.tile([64, 8], b16, tag="dsp")
            nc.gpsimd.dma_start(
                out=dsp[:, None, :],
                in_=t[64:65, :].rearrange("p (a b) -> p a b", a=64),
            )
            rsp = work.tile([64, 8], b16, tag="rsp")
            with nc.allow_low_precision(reason="bf16 softmax recip"):
                nc.vector.reciprocal(rsp[:], dsp[:])
            nc.gpsimd.dma_start(
                out=rb0[0:1, :].rearrange("p (a b) -> p a b", a=64),
                in_=rsp[:, None, :],
            )

        bcss = {}

        def bcast(c, j):
            rb0 = rb0e if j == 0 else rb0o
            bcs = work.tile([64, 512], b16, tag="bcs" + str(j), name="bcs")
            nc.gpsimd.partition_broadcast(bcs[:], rb0[0:1, :], channels=64)
            bcss[(c, j)] = bcs

        def norm_head(c, j):
            t = aun[c][j]
            bcs = bcss.pop((c, j))
            if j == 0:
                nc.vector.tensor_tensor(atn[0:64, c, :], t[0:64, :], bcs[:],
                                        MUL)
            else:
                so = work.tile([64, 512], f8, tag="so", name="so")
                nc.vector.tensor_tensor(so[:], t[0:64, :], bcs[:], MUL)
                nc.gpsimd.dma_start(out=atn[64:128, c, :], in_=so[:])
                aun.pop(c)

        def tail(b, step):
            if step == 0:
                pv_mm(b, 7)
            elif step == 1:
                pv_evict(b, 0)
            elif step == 2:
                pv_evict(b, 1)
                recip_spread(b, 0)
            elif step == 3:
                recip_spread(b, 1)
                bcast(b, 0)
            elif step == 4:
                bcast(b, 1)
            elif step == 5:
                norm_head(b, 0)
            elif step == 6:
                norm_head(b, 1)

        def post_tile(i):
            sc, dc = i // 2, i % 2
            dsl = slice(dc * 512, (dc + 1) * 512)
            pp = psA.tile([128, 512], f32, tag="pp")
            for u in range(4):
                nc.tensor.matmul(pp[:],
                                 lhsT=atn[:, 2 * u:2 * u + 2,
                                          sc * 128:(sc + 1) * 128],
                                 rhs=wp[:, 2 * u:2 * u + 2, dsl],
                                 start=(u == 0), stop=False, perf_mode=DR)
            nc.tensor.matmul(pp[:], lhsT=id128[:], rhs=xres[:, sc, dsl],
                             start=False, stop=True)
            ores = work.tile([128, 512], f32, tag="or", bufs=3, name="ores")
            nc.vector.tensor_scalar_mul(ores[:], pp[:], OSC)
            nc.sync.dma_start(out=out_d[sc, :, dsl], in_=ores[:])

        def pair_block(c):
            for step in range(16):
                if c < 8:
                    scores_exp(c, step)
                    if step >= 3 and step % 2 == 1:
                        pv_mm(c, (step - 3) // 2)
                if c >= 1:
                    tail(c - 1, step)
                if c == 0 and step < 12:
                    v_proj(step + 4)
                if c == 1 and step in (2, 6, 10, 14):
                    k_proj_tile(1, (step - 2) // 4)
                if c <= 5 and step == 9:
                    q_proj(c + 2)

        q_proj(0)
        q_proj(1)
        for tb in range(4):
            k_proj_tile(0, tb)
        for tcc in range(4):
            v_proj(tcc)
        for c in range(8):
            pair_block(c)
        pair_block(8)  # drain: tail of pair 7
        for i in range(8):
            post_tile(i)

    nc.compile()
    return nc


def get_program():
    if "nc" not in _prog_cache:
        _prog_cache["nc"] = _build_program()
    return _prog_cache["nc"]


def _chunk128(a):
    n = a.shape[1]
    return np.ascontiguousarray(a.reshape(8, 128, n).transpose(1, 0, 2))


def make_in_maps(X, Wq, Wk, Wv, Wpost):
    X = np.asarray(X, dtype=np.float32)
    wq_p = _chunk128(np.asarray(Wq, dtype=np.float32) * WSCALE).astype(F8)
    wk_p = _chunk128(np.asarray(Wk, dtype=np.float32) * WSCALE).astype(F8)
    wv_p = _chunk128(np.asarray(Wv, dtype=np.float32) * WSCALE).astype(F8)
    wp_p = _chunk128(np.asarray(Wpost, dtype=np.float32) * WSCALE).astype(F8)
    ktz = np.zeros((128, 8, S), dtype=F8)
    idm = (np.eye(128, dtype=np.float32) * (WSCALE * WSCALE)).astype(BF16)

    xt_b = []
    for b in range(B):
        xt_b.append(_chunk128(np.ascontiguousarray(X[b].T)).astype(F8))

    in_maps = []
    for core in range(NCORES):
        b = core // CORES_PER_BATCH
        q0 = (core % CORES_PER_BATCH) * SLOC
        xt = xt_b[b]
        xres = np.ascontiguousarray(
            X[b, q0:q0 + SLOC].reshape(4, 128, D).transpose(1, 0, 2)
        ).astype(BF16)
        in_maps.append(
            {
                "XT8": xt,
                "XTQ8": np.ascontiguousarray(xt[:, :, q0:q0 + SLOC]),
                "XRES": xres,
                "WQ8": wq_p,
                "WK8": wk_p,
                "WV8": wv_p,
                "WP8": wp_p,
                "KTZ8": ktz,
                "ID": idm,
            }
        )
    return in_maps


def assemble_output(results):
    out = np.empty((B, S, D), dtype=np.float32)
    for core, r in enumerate(results):
        b = core // CORES_PER_BATCH
        q0 = (core % CORES_PER_BATCH) * SLOC
        out[b, q0:q0 + SLOC] = np.asarray(r["OUT"]).reshape(SLOC, D)
    return out


def kernel(X, Wq, Wk, Wv, Wpost, _trace=False):
    from concourse.bass_utils import run_bass_kernel_spmd

    nc = get_program()
    in_maps = make_in_maps(X, Wq, Wk, Wv, Wpost)
    res = run_bass_kernel_spmd(nc, in_maps, core_ids=list(range(NCORES)), trace=_trace)
    out = assemble_output(res.results)
    if _trace:
        return out, res
    return out


# revision 10
# speedup vs baseline: 1.9194x; 1.0022x over previous
"""GroupedQueryAttentionLayer on 8 trn2 NeuronCores (Bass/Tile, SPMD).

Sharding: data-parallel over query rows; no collectives. Core i handles
batch b = i//4, query rows q0 = (i%4)*512 .. +512. Each core recomputes
its batch's K/V projection; outputs are disjoint row-slices.

v2.2: fp8e4 matmuls everywhere. DoubleRow (2x contraction packing ->
half the accumulation instructions; 1.0 cyc/row on this HW) for the
K/Q/V projections, PV, and post; plain fp8 for scores (contraction is
only 128 there, zero-padded K parity slots give per-head masking).
Softmax exp split between ACT (exact exp, fp8 out) and DVE (Schraudolph
affine -> int8 bits -> bitcast fp8, DVE_EXP_STEPS tcc's). Weights scaled
x16 on host (fp8 subnormal avoidance), compensated in the exp scale
(1/2048) and the post-evict scale 1/256 (Wpost x16 fp8; residual added
in PSUM via a 256*I identity matmul). Softmax denominator from a ones
column in the V weight tile; reciprocal via partition-spread DMAs; recip
broadcast across partitions via gpsimd partition_broadcast. Keep the SP
(sync) engine lean -- loading it with small DMA dispatches serializes
the whole pipeline (measured: 199us -> 382us).

Host layouts (fp8 = ml_dtypes.float8_e4m3):
  XT8  [128, 8, 2048]  X[b].T k-chunked      XTQ8 [128, 8, 512] q-slab cols
  XRES [128, 4, 1024]  bf16 residual rows    ID   [128, 128] bf16 256*I
  WQ8/WK8/WV8/WP8      k-chunked weights x16 (fp8)
  KTZ8 [128, 8, 2048]  zeros (K parity slots runtime-written)
Output OUT [4, 128, 1024] f32, OUT[sc, p, :] = row q0 + sc*128 + p.
"""

import math

import numpy as np
import ml_dtypes

BF16 = ml_dtypes.bfloat16
F8 = ml_dtypes.float8_e4m3

B, S, D = 2, 2048, 1024
HEADS, GROUPS, E = 16, 4, 64
NCORES = 8
CORES_PER_BATCH = NCORES // B
SLOC = B * S // NCORES
WSCALE = 16.0
EXPSC = (1.0 / math.sqrt(E)) / (WSCALE * WSCALE)
SCH_A = EXPSC * 8.0 / math.log(2.0)
SCH_B = 55.4
OSC = 1.0 / (WSCALE * WSCALE)  # post evict scale (atn x16 * wp x16)

# exp tcc-steps handled by DVE (rest on ACT)
DVE_EXP_STEPS = (0, 3, 6, 8, 11, 14)

_prog_cache = {}


def _build_program():
    from contextlib import ExitStack

    import concourse.bacc as bacc
    import concourse.tile as tile
    from concourse import mybir

    f32 = mybir.dt.float32
    b16 = mybir.dt.bfloat16
    f8 = mybir.dt.float8e4
    i8 = mybir.dt.int8
    Exp = mybir.ActivationFunctionType.Exp
    DR = mybir.MatmulPerfMode.DoubleRow
    MUL = mybir.AluOpType.mult
    ADD = mybir.AluOpType.add

    nc = bacc.Bacc("TRN2", target_bir_lowering=False)

    xt_d = nc.dram_tensor("XT8", [128, 8, S], f8, kind="ExternalInput")
    xtq_d = nc.dram_tensor("XTQ8", [128, 8, SLOC], f8, kind="ExternalInput")
    xres_d = nc.dram_tensor("XRES", [128, 4, D], b16, kind="ExternalInput")
    wq_d = nc.dram_tensor("WQ8", [128, 8, 1024], f8, kind="ExternalInput")
    wk_d = nc.dram_tensor("WK8", [128, 8, 256], f8, kind="ExternalInput")
    wv_d = nc.dram_tensor("WV8", [128, 8, 256], f8, kind="ExternalInput")
    wp_d = nc.dram_tensor("WP8", [128, 8, 1024], f8, kind="ExternalInput")
    ktz_d = nc.dram_tensor("KTZ8", [128, 8, S], f8, kind="ExternalInput")
    id_d = nc.dram_tensor("ID", [128, 128], b16, kind="ExternalInput")
    out_d = nc.dram_tensor("OUT", [4, 128, D], f32, kind="ExternalOutput")

    with tile.TileContext(nc) as tc, ExitStack() as ctx:
        consts = ctx.enter_context(tc.tile_pool(name="consts", bufs=1))
        work = ctx.enter_context(tc.tile_pool(name="work", bufs=2))
        psS = ctx.enter_context(tc.tile_pool(name="psS", bufs=2, space="PSUM"))
        psV = ctx.enter_context(tc.tile_pool(name="psV", bufs=1, space="PSUM"))
        psA = ctx.enter_context(tc.tile_pool(name="psA", bufs=2, space="PSUM"))

        xt = consts.tile([128, 8, S], f8)
        xtq = consts.tile([128, 8, SLOC], f8)
        xres = consts.tile([128, 4, D], b16)
        wq = consts.tile([128, 8, 1024], f8)
        wk = consts.tile([128, 8, 256], f8)
        wv = consts.tile([128, 8, 256], f8)
        wp = consts.tile([128, 8, 1024], f8)
        qt = consts.tile([128, 8, SLOC], f8)
        ktz = consts.tile([128, 8, S], f8)
        ksh = consts.tile([128, 2, S], f8)  # K eviction shadow
        id128 = consts.tile([128, 128], b16)
        vpr = consts.tile([128, 16, 4, 128], f8)  # V(64) | ones | zeros(63)
        atn = consts.tile([128, 8, SLOC], f8)
        rb0e = consts.tile([1, SLOC], b16)
        rb0o = consts.tile([1, SLOC], b16)
        warm = consts.tile([128, 8], f32)

        nc.vector.memset(warm[:], 0.0)
        nc.scalar.activation(warm[:], warm[:], Exp)  # exp table preload
        nc.vector.memset(vpr[:, :, :, 64:65], 1.0)
        nc.vector.memset(vpr[:, :, :, 65:128], 0.0)

        nc.sync.dma_start(out=wq[:], in_=wq_d[:])
        nc.sync.dma_start(out=xtq[:], in_=xtq_d[:])
        nc.sync.dma_start(out=wk[:], in_=wk_d[:])
        nc.sync.dma_start(out=xt[:, :, 0:512], in_=xt_d[:, :, 0:512])
        nc.sync.dma_start(out=ktz[:, 0:4, :], in_=ktz_d[:, 0:4, :])
        nc.sync.dma_start(out=wv[:], in_=wv_d[:])
        nc.sync.dma_start(out=xt[:, :, 512:1024], in_=xt_d[:, :, 512:1024])
        nc.sync.dma_start(out=xt[:, :, 1024:1536], in_=xt_d[:, :, 1024:1536])
        nc.sync.dma_start(out=xt[:, :, 1536:2048], in_=xt_d[:, :, 1536:2048])
        nc.sync.dma_start(out=ktz[:, 4:8, :], in_=ktz_d[:, 4:8, :])
        nc.sync.dma_start(out=wp[:], in_=wp_d[:])
        nc.sync.dma_start(out=xres[:], in_=xres_d[:])
        nc.sync.dma_start(out=id128[:], in_=id_d[:])

        def q_proj(c):
            ps = psA.tile([128, SLOC], f32, tag="pp")
            for u in range(4):
                nc.tensor.matmul(
                    ps[:],
                    lhsT=wq[:, 2 * u:2 * u + 2, c * 128:(c + 1) * 128],
                    rhs=xtq[:, 2 * u:2 * u + 2, :],
                    start=(u == 0), stop=(u == 3), perf_mode=DR,
                )
            nc.vector.tensor_copy(qt[:, c, :], ps[:])

        def k_proj_tile(gg, tb):
            """K groups (2gg, 2gg+1), t-block tb -> shadow -> ktz slots."""
            sl = slice(tb * 512, (tb + 1) * 512)
            ps = psA.tile([128, 512], f32, tag="pp")
            for u in range(4):
                nc.tensor.matmul(
                    ps[:],
                    lhsT=wk[:, 2 * u:2 * u + 2, gg * 128:(gg + 1) * 128],
                    rhs=xt[:, 2 * u:2 * u + 2, sl],
                    start=(u == 0), stop=(u == 3), perf_mode=DR,
                )
            nc.vector.tensor_copy(ksh[:, gg, sl], ps[:])
            ga, gb = 2 * gg, 2 * gg + 1
            nc.gpsimd.dma_start(out=ktz[0:64, 2 * ga, sl], in_=ksh[0:64, gg, sl])
            nc.gpsimd.dma_start(out=ktz[64:128, 2 * ga + 1, sl],
                                in_=ksh[0:64, gg, sl])
            nc.gpsimd.dma_start(out=ktz[64:128, 2 * gb + 1, sl],
                                in_=ksh[64:128, gg, sl])
            nc.gpsimd.dma_start(out=ktz[0:64, 2 * gb, sl],
                                in_=ksh[64:128, gg, sl])

        def v_proj(tcc):
            ps = psA.tile([128, 256], f32, tag="pp")
            for u in range(4):
                nc.tensor.matmul(
                    ps[:],
                    lhsT=xt[:, 2 * u:2 * u + 2, tcc * 128:(tcc + 1) * 128],
                    rhs=wv[:, 2 * u:2 * u + 2, :],
                    start=(u == 0), stop=(u == 3), perf_mode=DR,
                )
            nc.vector.tensor_copy(
                vpr[:, tcc, :, 0:64], ps.rearrange("p (g e) -> p g e", g=4)
            )

        exws = {}
        pvt = {}
        aun = {}

        def scores_exp(c, tcc):
            g = c // 2
            ps2 = psS.tile([128, 2, 512], f32, tag="sc", name="ps2")
            for j in range(2):
                nc.tensor.matmul(
                    ps2[:, j, :],
                    lhsT=ktz[:, 2 * g + j, tcc * 128:(tcc + 1) * 128],
                    rhs=qt[:, c, :], start=True, stop=True)
            u = tcc // 2
            if tcc % 2 == 0:
                exws[(c, u)] = work.tile([128, 2, 2, 512], i8, tag="exw",
                                         bufs=5, name="exw")
            exw = exws[(c, u)]
            dst = exw[:, tcc % 2, :, :]
            if tcc in DVE_EXP_STEPS:
                nc.vector.tensor_scalar(dst, ps2[:], SCH_A, SCH_B, MUL, ADD)
            else:
                nc.scalar.activation(dst.bitcast(f8), ps2[:], Exp, scale=EXPSC)

        def pv_mm(c, u):
            g = c // 2
            if u == 0:
                pvt[c] = (
                    psV.tile([128, 512], f32, tag="pve", name="pve"),
                    psV.tile([128, 512], f32, tag="pvo", name="pvo"),
                )
            pve, pvo = pvt[c]
            exw = exws.pop((c, u))
            for j, pv in ((0, pve), (1, pvo)):
                nc.tensor.matmul(
                    pv[:],
                    lhsT=vpr[:, 2 * u:2 * u + 2, g, :],
                    rhs=exw[:, :, j, :].bitcast(f8),
                    start=(u == 0), stop=(u == 7), perf_mode=DR,
                )

        def pv_evict(c, j):
            t = work.tile([65, 512], b16, tag="aun" + str(j), name="aun")
            nc.vector.tensor_copy(t[:], pvt[c][j][0:65, :])
            if j == 0:
                aun[c] = [t, None]
            else:
                aun[c][1] = t
                pvt.pop(c)

        def recip_spread(c, j):
            """den row 64 of te/to -> [64,8] -> recip -> rb0 row 0."""
            t = aun[c][j]
            rb0 = rb0e if j == 0 else rb0o
            dsp = work.tile([64, 8], b16, tag="dsp")
            nc.gpsimd.dma_start(
                out=dsp[:, None, :],
                in_=t[64:65, :].rearrange("p (a b) -> p a b", a=64),
            )
            rsp = work.tile([64, 8], b16, tag="rsp")
            with nc.allow_low_precision(reason="bf16 softmax recip"):
                nc.vector.reciprocal(rsp[:], dsp[:])
            nc.gpsimd.dma_start(
                out=rb0[0:1, :].rearrange("p (a b) -> p a b", a=64),
                in_=rsp[:, None, :],
            )

        bcss = {}

        def bcast(c, j):
            rb0 = rb0e if j == 0 else rb0o
            bcs = work.tile([64, 512], b16, tag="bcs" + str(j), name="bcs")
            nc.gpsimd.partition_broadcast(bcs[:], rb0[0:1, :], channels=64)
            bcss[(c, j)] = bcs

        def norm_head(c, j):
            t = aun[c][j]
            bcs = bcss.pop((c, j))
            if j == 0:
                nc.vector.tensor_tensor(atn[0:64, c, :], t[0:64, :], bcs[:],
                                        MUL)
            else:
                so = work.tile([64, 512], f8, tag="so", name="so")
                nc.vector.tensor_tensor(so[:], t[0:64, :], bcs[:], MUL)
                nc.gpsimd.dma_start(out=atn[64:128, c, :], in_=so[:])
                aun.pop(c)

        def tail(b, step):
            if step == 0:
                pv_mm(b, 7)
            elif step == 1:
                pv_evict(b, 0)
            elif step == 2:
                pv_evict(b, 1)
                recip_spread(b, 0)
            elif step == 3:
                recip_spread(b, 1)
                bcast(b, 0)
            elif step == 4:
                bcast(b, 1)
            elif step == 5:
                norm_head(b, 0)
            elif step == 6:
                norm_head(b, 1)

        def post_tile(i):
            sc, dc = i // 2, i % 2
            dsl = slice(dc * 512, (dc + 1) * 512)
            pp = psA.tile([128, 512], f32, tag="pp")
            for u in range(4):
                nc.tensor.matmul(pp[:],
                                 lhsT=atn[:, 2 * u:2 * u + 2,
                                          sc * 128:(sc + 1) * 128],
                                 rhs=wp[:, 2 * u:2 * u + 2, dsl],
                                 start=(u == 0), stop=False, perf_mode=DR)
            nc.tensor.matmul(pp[:], lhsT=id128[:], rhs=xres[:, sc, dsl],
                             start=False, stop=True)
            ores = work.tile([128, 512], f32, tag="or", bufs=3, name="ores")
            nc.vector.tensor_scalar_mul(ores[:], pp[:], OSC)
            nc.sync.dma_start(out=out_d[sc, :, dsl], in_=ores[:])

        def pair_block(c):
            for step in range(16):
                if c < 8:
                    scores_exp(c, step)
                    if step >= 3 and step % 2 == 1:
                        pv_mm(c, (step - 3) // 2)
                if c >= 1:
                    tail(c - 1, step)
                if c == 0 and step < 12:
                    v_proj(step + 4)
                if c == 1 and step in (2, 6, 10, 14):
                    k_proj_tile(1, (step - 2) // 4)
                if c <= 5 and step == 9:
                    q_proj(c + 2)

        q_proj(0)
        q_proj(1)
        for tb in range(4):
            k_proj_tile(0, tb)
        for tcc in range(4):
            v_proj(tcc)
        for c in range(8):
            pair_block(c)
        pair_block(8)  # drain: tail of pair 7
        for i in range(8):
            post_tile(i)

    nc.compile()
    return nc


def get_program():
    if "nc" not in _prog_cache:
        _prog_cache["nc"] = _build_program()
    return _prog_cache["nc"]


def _chunk128(a):
    n = a.shape[1]
    return np.ascontiguousarray(a.reshape(8, 128, n).transpose(1, 0, 2))


def make_in_maps(X, Wq, Wk, Wv, Wpost):
    X = np.asarray(X, dtype=np.float32)
    wq_p = _chunk128(np.asarray(Wq, dtype=np.float32) * WSCALE).astype(F8)
    wk_p = _chunk128(np.asarray(Wk, dtype=np.float32) * WSCALE).astype(F8)
    wv_p = _chunk128(np.asarray(Wv, dtype=np.float32) * WSCALE).astype(F8)
    wp_p = _chunk128(np.asarray(Wpost, dtype=np.float32) * WSCALE).astype(F8)
    ktz = np.zeros((128, 8, S), dtype=F8)
    idm = (np.eye(128, dtype=np.float32) * (WSCALE * WSCALE)).astype(BF16)

    xt_b = []
    for b in range(B):
        xt_b.append(_chunk128(np.ascontiguousarray(X[b].T)).astype(F8))

    in_maps = []
    for core in range(NCORES):
        b = core // CORES_PER_BATCH
        q0 = (core % CORES_PER_BATCH) * SLOC
        xt = xt_b[b]
        xres = np.ascontiguousarray(
            X[b, q0:q0 + SLOC].reshape(4, 128, D).transpose(1, 0, 2)
        ).astype(BF16)
        in_maps.append(
            {
                "XT8": xt,
                "XTQ8": np.ascontiguousarray(xt[:, :, q0:q0 + SLOC]),
                "XRES": xres,
                "WQ8": wq_p,
                "WK8": wk_p,
                "WV8": wv_p,
                "WP8": wp_p,
                "KTZ8": ktz,
                "ID": idm,
            }
        )
    return in_maps


def assemble_output(results):
    out = np.empty((B, S, D), dtype=np.float32)
    for core, r in enumerate(results):
        b = core // CORES_PER_BATCH
        q0 = (core % CORES_PER_BATCH) * SLOC
        out[b, q0:q0 + SLOC] = np.asarray(r["OUT"]).reshape(SLOC, D)
    return out


def kernel(X, Wq, Wk, Wv, Wpost, _trace=False):
    from concourse.bass_utils import run_bass_kernel_spmd

    nc = get_program()
    in_maps = make_in_maps(X, Wq, Wk, Wv, Wpost)
    res = run_bass_kernel_spmd(nc, in_maps, core_ids=list(range(NCORES)), trace=_trace)
    out = assemble_output(res.results)
    if _trace:
        return out, res
    return out
